# revision 17
# baseline (speedup 1.0000x reference)
"""Bass/Trainium2 kernel for per-head attention (B=2, S=2048, H=12, DM=768, DH=64).

Sharding: 24 (batch, head) pairs -> 8 cores x 3 pairs. Host pre-transposes the
per-pair activations to [DM, S] (and casts to fp16) so the device reads
contiguous [128, 2048] tiles with d_model on partitions (matmul contraction
dim). The device computes attn_out^T [DM, S] fp32 per pair; the host
transposes back.

Matmul operands are fp16 (full PE rate, warms the HAM clock gate); PSUM
accumulation is fp32. fp16's 10-bit mantissa and the small value ranges here
keep the end-to-end error ~5e-4.

Per pair:
  Q^T = W_Q^T X_q^T   [64, S]  (accumulated over 6 d_model chunks; biases are
  structurally zero in this problem -- a bias path exists and auto-enables)
  K^T, V^T likewise; V natural [S, 64] via 16 PE transposes of V^T blocks,
  plus a DMA'd ones column -> Vaug [128, 65] per sk tile.
  scores^T[sk, sq] block = (K^T block).T @ Q^T group   [128, 512]
  P_u = exp(0.125 * scores^T)  (no max subtraction: |scores| < ~3, exp safe),
  masked to exact 0 above the diagonal via 4 precomputed 0/1 masks; blocks
  entirely above the diagonal are skipped.
  Zaug = sum_sk Vaug.T @ P_u  [65, 512]: rows 0:64 unnormalized Z^T, row 64 =
  softmax denominators.
  o = (WOaug block).T @ Zaug with WOaug = [W_O[h]; b_O/H]; the final
  PSUM->SBUF eviction multiplies by broadcast(1/denom), giving
  attn_out^T = Z W_O + b_O/H exactly.
  The outproj of group g is emitted after the scores/Z of group g+1 so the
  reciprocal/broadcast chain never stalls the PE (stalls > 3.4us would
  re-throttle the PE clock to 1.2 GHz).
"""

import numpy as np

B, S, H, DM, DH = 2, 2048, 12, 768, 64
P = 128
NCORES = 8
PPC = (B * H) // NCORES   # pairs per core = 3
NCH = DM // P             # 6 d_model chunks
NG = 4                    # sq groups
GW = S // NG              # 512
NSK = S // P              # 16 sk tiles
VW = DH + 1               # 65 (V augmented with ones column)

NP_IN = np.float16

_NC_CACHE = {}


def _build_bass(use_bias):
    import concourse.mybir as mybir
    import concourse.tile as tile
    from concourse import bacc
    from contextlib import ExitStack

    dt = mybir.dt
    f32 = dt.float32
    f16 = dt.float16
    AF = mybir.ActivationFunctionType

    nc = bacc.Bacc("TRN2", target_bir_lowering=False, debug=False)

    xq = nc.dram_tensor("xqT", [PPC, NCH, P, S], f16, kind="ExternalInput").ap()
    xk = nc.dram_tensor("xkT", [PPC, NCH, P, S], f16, kind="ExternalInput").ap()
    xv = nc.dram_tensor("xvT", [PPC, NCH, P, S], f16, kind="ExternalInput").ap()
    wq = nc.dram_tensor("wq", [PPC, NCH, P, DH], f16, kind="ExternalInput").ap()
    wk = nc.dram_tensor("wk", [PPC, NCH, P, DH], f16, kind="ExternalInput").ap()
    wv = nc.dram_tensor("wv", [PPC, NCH, P, DH], f16, kind="ExternalInput").ap()
    if use_bias:
        bq = nc.dram_tensor("bq", [PPC, 1, DH], f16, kind="ExternalInput").ap()
        bk = nc.dram_tensor("bk", [PPC, 1, DH], f16, kind="ExternalInput").ap()
        bv = nc.dram_tensor("bv", [PPC, 1, DH], f16, kind="ExternalInput").ap()
        onesr = nc.dram_tensor(
            "ones_row", [1, GW], f16, kind="ExternalInput").ap()
    wo = nc.dram_tensor("wo", [PPC, VW, DM], f16, kind="ExternalInput").ap()
    mk = nc.dram_tensor("masks", [NG, P, GW], f16, kind="ExternalInput").ap()
    onesc = nc.dram_tensor("ones_col", [P, NSK, 1], f16, kind="ExternalInput").ap()
    idin = nc.dram_tensor("ident64", [DH, DH], f16, kind="ExternalInput").ap()
    outT = nc.dram_tensor("outT", [PPC, NCH, P, S], f32, kind="ExternalOutput").ap()

    with tile.TileContext(nc) as tc, ExitStack() as ctx:
        consts = ctx.enter_context(tc.tile_pool(name="consts", bufs=1))
        wpool = ctx.enter_context(tc.tile_pool(name="wpool", bufs=2))
        xin = ctx.enter_context(tc.tile_pool(name="xin", bufs=8))
        prj = ctx.enter_context(tc.tile_pool(name="prj", bufs=2))
        expp = ctx.enter_context(tc.tile_pool(name="expp", bufs=4))
        smal = ctx.enter_context(tc.tile_pool(name="smal", bufs=4))
        obuf = ctx.enter_context(tc.tile_pool(name="obuf", bufs=2))
        ps_prj = ctx.enter_context(tc.tile_pool(name="ps_prj", bufs=1, space="PSUM"))
        ps_att = ctx.enter_context(tc.tile_pool(name="ps_att", bufs=4, space="PSUM"))

        ident = consts.tile([DH, DH], f16)
        nc.sync.dma_start(ident[:], idin)
        masks = consts.tile([P, NG * GW], f16)
        nc.sync.dma_start(
            masks[:].rearrange("p (j c) -> p j c", j=NG),
            mk.rearrange("j p c -> p j c"),
        )
        if use_bias:
            ones = consts.tile([1, GW], f16)
            nc.sync.dma_start(ones[:], onesr)

        # outproj work queue: carried across groups AND pairs so the PE
        # always has matmul work while recip/broadcast chains complete
        pending = []

        def flush_outproj():
            zaug_, bc_, p_, g_, wo_sb_ = pending.pop(0)
            gs_ = slice(g_ * GW, (g_ + 1) * GW)
            ob = obuf.tile([P, NCH * GW], f32, tag="ob")
            for t in range(NCH):
                o_ps = ps_att.tile([P, GW], f32, tag="att")
                nc.tensor.matmul(
                    o_ps[:],
                    lhsT=wo_sb_[:, t * P:(t + 1) * P],
                    rhs=zaug_[:],
                    start=True,
                    stop=True,
                )
                nc.vector.tensor_mul(
                    ob[:, t * GW:(t + 1) * GW], o_ps[:], bc_[:]
                )
            nc.sync.dma_start(
                outT[p_].rearrange("t q c -> q t c")[:, :, gs_],
                ob[:].rearrange("q (t c) -> q t c", t=NCH),
            )

        for p in range(PPC):
            # per-pair weights
            wq_sb = wpool.tile([P, NCH * DH], f16, tag="wq")
            nc.sync.dma_start(
                wq_sb[:].rearrange("p (c e) -> p c e", c=NCH),
                wq[p].rearrange("c p e -> p c e"),
            )
            wk_sb = wpool.tile([P, NCH * DH], f16, tag="wk")
            nc.sync.dma_start(
                wk_sb[:].rearrange("p (c e) -> p c e", c=NCH),
                wk[p].rearrange("c p e -> p c e"),
            )
            wv_sb = wpool.tile([P, NCH * DH], f16, tag="wv")
            nc.sync.dma_start(
                wv_sb[:].rearrange("p (c e) -> p c e", c=NCH),
                wv[p].rearrange("c p e -> p c e"),
            )
            if use_bias:
                bq_sb = wpool.tile([1, DH], f16, tag="bq")
                nc.sync.dma_start(bq_sb[:], bq[p])
                bk_sb = wpool.tile([1, DH], f16, tag="bk")
                nc.sync.dma_start(bk_sb[:], bk[p])
                bv_sb = wpool.tile([1, DH], f16, tag="bv")
                nc.sync.dma_start(bv_sb[:], bv[p])
            wo_sb = wpool.tile([VW, DM], f16, tag="wo")
            nc.sync.dma_start(wo_sb[:], wo[p])

            # --- projections: out^T = W^T X^T (+ b x 1), accumulated chunks
            def project(xdram, w_sb, b_sb, tag):
                pt_ps = ps_prj.tile([DH, S], f32, tag="prj")
                for c in range(NCH):
                    x_c = xin.tile([P, S], f16, tag="xin")
                    nc.sync.dma_start(x_c[:], xdram[p, c])
                    for g in range(NG):
                        gs = slice(g * GW, (g + 1) * GW)
                        nc.tensor.matmul(
                            pt_ps[:, gs],
                            lhsT=w_sb[:, c * DH:(c + 1) * DH],
                            rhs=x_c[:, gs],
                            start=(c == 0),
                            stop=(c == NCH - 1) and not use_bias,
                        )
                if use_bias:
                    for g in range(NG):
                        gs = slice(g * GW, (g + 1) * GW)
                        nc.tensor.matmul(
                            pt_ps[:, gs], lhsT=b_sb[:], rhs=ones[:],
                            start=False, stop=True,
                        )
                # split eviction across ACT and DVE to halve the PSUM-slot
                # turnaround stall before the next projection's matmuls
                pt_sb = prj.tile([DH, S], f16, tag=tag)
                nc.scalar.copy(pt_sb[:, 0:S // 2], pt_ps[:, 0:S // 2])
                nc.vector.tensor_copy(pt_sb[:, S // 2:S], pt_ps[:, S // 2:S])
                return pt_sb

            vt_sb = project(xv, wv_sb, bv_sb if use_bias else None, "vt")
            # flush the previous pair's carried outprojs here: their
            # reciprocal chains finished long ago, and this keeps the PE
            # dense between the V projection and the transposes below
            while len(pending) > 0:
                flush_outproj()
            # V transposes overlap the Q/K projection matmuls below
            vaug = prj.tile([P, NSK * VW], f16, tag="vaug")
            nc.sync.dma_start(
                vaug[:].rearrange("p (i w) -> p i w", w=VW)[:, :, DH:VW], onesc
            )
            for i in range(NSK):
                tp_ps = ps_att.tile([P, DH], f16, tag="att")
                nc.tensor.transpose(
                    tp_ps[:], vt_sb[:, i * P:(i + 1) * P], ident[:]
                )
                nc.vector.tensor_copy(vaug[:, i * VW:i * VW + DH], tp_ps[:])

            qt_sb = project(xq, wq_sb, bq_sb if use_bias else None, "qt")
            kt_sb = project(xk, wk_sb, bk_sb if use_bias else None, "kt")

            for g in range(NG):
                gs = slice(g * GW, (g + 1) * GW)
                nsk = 4 * (g + 1)
                z_ps = ps_att.tile([VW, GW], f32, tag="att")

                def emit_scores(i):
                    s_ps = ps_att.tile([P, GW], f32, tag="att")
                    nc.tensor.matmul(
                        s_ps[:],
                        lhsT=kt_sb[:, i * P:(i + 1) * P],
                        rhs=qt_sb[:, gs],
                        start=True,
                        stop=True,
                    )
                    e_sb = expp.tile([P, GW], f16, tag="exp")
                    nc.scalar.activation(e_sb[:], s_ps[:], AF.Exp, scale=0.125)
                    if i >= 4 * g:
                        j = i - 4 * g
                        em_sb = expp.tile([P, GW], f16, tag="exp")
                        nc.vector.tensor_mul(
                            em_sb[:], e_sb[:], masks[:, j * GW:(j + 1) * GW]
                        )
                        return em_sb
                    return e_sb

                def emit_z(i, e_use):
                    nc.tensor.matmul(
                        z_ps[:],
                        lhsT=vaug[:, i * VW:(i + 1) * VW],
                        rhs=e_use[:],
                        start=(i == 0),
                        stop=(i == nsk - 1),
                    )

                # z(i) is emitted after scores(i+1) so the in-order PE queue
                # never stalls on the exp/mask chain of block i
                e_prev = emit_scores(0)
                for i in range(1, nsk):
                    e_cur = emit_scores(i)
                    emit_z(i - 1, e_prev)
                    e_prev = e_cur
                emit_z(nsk - 1, e_prev)

                # evict unnormalized Zaug (outproj input); reciprocal and its
                # broadcast run on DVE/GpSimd off the PE critical path
                zaug = smal.tile([VW, GW], f16, tag="zaug")
                nc.scalar.copy(zaug[:], z_ps[:])
                recip = smal.tile([1, GW], f32, tag="recip")
                nc.vector.reciprocal(recip[:], z_ps[DH:VW, :])
                bc = smal.tile([P, GW], f32, tag="bc")
                nc.gpsimd.partition_broadcast(bc[:], recip[:])
                pending.append((zaug, bc, p, g, wo_sb))
                if len(pending) > 2:
                    flush_outproj()
        while pending:
            flush_outproj()

    nc.compile()
    return nc


def get_nc(use_bias=False):
    if use_bias not in _NC_CACHE:
        _NC_CACHE[use_bias] = _build_bass(use_bias)
    return _NC_CACHE[use_bias]


def _pairs_for_core(c):
    return [(idx // H, idx % H) for idx in range(c * PPC, (c + 1) * PPC)]


def make_masks():
    # mask[j, p, f] = 1.0 iff key pos 128*j + p <= query pos f (within block)
    j = np.arange(NG)[:, None, None]
    p = np.arange(P)[None, :, None]
    f = np.arange(GW)[None, None, :]
    return (f >= P * j + p).astype(NP_IN)


def make_in_maps(inputs, use_bias):
    xq = np.asarray(inputs["normalized_resid_pre_q"], dtype=np.float32)
    xk = np.asarray(inputs["normalized_resid_pre_k"], dtype=np.float32)
    xv = np.asarray(inputs["normalized_resid_pre_v"], dtype=np.float32)
    W_Q = np.asarray(inputs["W_Q"], dtype=np.float32)
    W_K = np.asarray(inputs["W_K"], dtype=np.float32)
    W_V = np.asarray(inputs["W_V"], dtype=np.float32)
    b_Q = np.asarray(inputs["b_Q"], dtype=np.float32)
    b_K = np.asarray(inputs["b_K"], dtype=np.float32)
    b_V = np.asarray(inputs["b_V"], dtype=np.float32)
    W_O = np.asarray(inputs["W_O"], dtype=np.float32)
    b_O = np.asarray(inputs["b_O"], dtype=np.float32)

    masks = make_masks()
    onesc = np.ones((P, NSK, 1), NP_IN)
    ident64 = np.eye(DH, dtype=NP_IN)
    in_maps = []
    for c in range(NCORES):
        pairs = _pairs_for_core(c)
        m = {
            "xqT": np.stack(
                [xq[b, :, h, :].T.astype(NP_IN).reshape(NCH, P, S)
                 for b, h in pairs]),
            "xkT": np.stack(
                [xk[b, :, h, :].T.astype(NP_IN).reshape(NCH, P, S)
                 for b, h in pairs]),
            "xvT": np.stack(
                [xv[b, :, h, :].T.astype(NP_IN).reshape(NCH, P, S)
                 for b, h in pairs]),
            "wq": np.stack(
                [W_Q[h].astype(NP_IN).reshape(NCH, P, DH) for b, h in pairs]),
            "wk": np.stack(
                [W_K[h].astype(NP_IN).reshape(NCH, P, DH) for b, h in pairs]),
            "wv": np.stack(
                [W_V[h].astype(NP_IN).reshape(NCH, P, DH) for b, h in pairs]),
            "wo": np.stack(
                [np.concatenate([W_O[h], (b_O / H)[None, :]], axis=0).astype(NP_IN)
                 for b, h in pairs]),
            "masks": masks,
            "ones_col": onesc,
            "ident64": ident64,
        }
        if use_bias:
            m["bq"] = np.stack([b_Q[h][None, :].astype(NP_IN) for b, h in pairs])
            m["bk"] = np.stack([b_K[h][None, :].astype(NP_IN) for b, h in pairs])
            m["bv"] = np.stack([b_V[h][None, :].astype(NP_IN) for b, h in pairs])
            m["ones_row"] = np.ones((1, GW), NP_IN)
        in_maps.append(m)
    return in_maps


def needs_bias(inputs):
    return any(
        np.any(np.asarray(inputs[k])) for k in ("b_Q", "b_K", "b_V")
    )


def assemble_output(results):
    out = np.empty((B, S, H, DM), np.float32)
    for c in range(NCORES):
        for j, (b, h) in enumerate(_pairs_for_core(c)):
            out[b, :, h, :] = results[c]["outT"][j].reshape(DM, S).T
    return out


def kernel(**inputs):
    from concourse import bass_utils

    use_bias = needs_bias(inputs)
    nc = get_nc(use_bias)
    in_maps = make_in_maps(inputs, use_bias)
    res = bass_utils.run_bass_kernel_spmd(nc, in_maps, core_ids=list(range(NCORES)))
    return assemble_output(res.results)
